# revision 34
# baseline (speedup 1.0000x reference)
"""BertMoELayer (B=4, S=2048, H=768, F=3072, E=8, top-2) on 8 Trainium2 cores.

Expert-parallel: one expert per core. The host evaluates the router in fp32
(it must anyway, to decide the shard assignment matching jax.lax.top_k
tie-breaking) and also produces the top-2 softmax weights there — they are
O(T) scalars and bit-match the reference combine. The device runs the dense
FFN only:

  per core c, over its gathered tokens (capacity = max expert load, exact):
    hT      = gelu(WiT^T @ xT + bi[c])        (bf16 matmul, fp32 psum)
    outT_c  = WoT-chunk^T @ hT + bo[c]        (bf16 matmul, fp32 psum, bf16 out;
                                               [h, token] orientation so the
                                               tail tokens cost proportionally)

The host unshards by scatter-adding each core's expert rows scaled by its
fp32 routing weight.

Startup is the only non-roofline phase, so the DMA schedule is built around
the measured ring behavior: per-queue throughput ramps with per-partition
line size (2KB lines ~50 GB/s early, 6-18KB lines 230+ GB/s), the event-
semaphore pool that tracks DMA completion is small (~8) so injects beyond it
serialize on earlier completions, and each DMA_DIRECT2D costs ~0.7us of the
issuing engine. Hence:
  - x block 0 goes on the gpsimd ring (fastest ramp, nothing else to do) in
    two k-pieces so the first L1 matmul can start ~3us into the body;
  - wi streams in ramping group sizes (1,2,3,4 cols*128 on sync; 6,8 on
    gpsimd behind x0) so chain j's weights always land just ahead of it;
  - wo is packed C-MAJOR (output-chunk-major) and split scalar(c0-1)/
    gpsimd(c2-5), so layer 2 can start on a partial wo stream;
  - bi+bo ride one tiny scalar-ring DMA; scalar issues only 2 injects and
    is then free for the gelu chain from ~3us.

All tensors are HOST-PREPACKED into SBUF-partition-major layout ([128, ...]
with each partition's bytes contiguous in DRAM): every DMA moves 3-24KB
contiguous lines per partition.
"""

import numpy as np
import ml_dtypes

import concourse.bass as bass
import concourse.tile as tile
from concourse import bacc, mybir
from concourse.bass_utils import run_bass_kernel_spmd

B, S, H, F, E = 4, 2048, 768, 3072, 8
T = B * S
N_CORES = 8

P = 128          # SBUF partitions
KH = H // P      # 6   h-chunks
KF = F // P      # 24  f-chunks

F32 = mybir.dt.float32
BF16 = mybir.dt.bfloat16
BF16_NP = ml_dtypes.bfloat16

# wi column groups (in j units of 128): small groups interleaved across all
# three DGE queues in deadline order at startup (DMAs sharing a queue
# transfer CONCURRENTLY, pro packet size — big-packet bulk must not
# co-reside with these); the 6+7 bulk is paced onto the gpsimd queue
# mid-loop once gelu 0 has run.
WI_GROUPS = (1, 2, 2, 2, 2, 2, 6, 7)


def make_blocks(cap: int):
    """Token blocks: 512-token blocks (psum-bank-sized) with a 257..512
    tail; every block >= 256 tokens so L1 chains stay matmul-bound (a
    512-free bf16 matmul is 213ns vs ~97ns LDWEIGHTS)."""
    assert cap >= 512
    blocks = []
    rem = cap
    while rem > 768:
        blocks.append(512)
        rem -= 512
    if rem > 512:
        blocks.append(256)
        rem -= 256
    blocks.append(rem)
    assert sum(blocks) == cap
    assert all(b % 128 == 0 for b in blocks[:-1]) and blocks[-1] <= 512
    return blocks


def build_nc(cap: int):
    """Per-core program: dense expert FFN over `cap` tokens."""
    blocks = make_blocks(cap)
    nblk = len(blocks)

    # Bacc (not plain Bass): its compile() pass splits multi-wait instructions
    # into event-semaphore chains, which walrus requires (max 1 wait per inst).
    nc = bacc.Bacc(None)

    # All inputs prepacked on host to [128 partitions, contiguous bytes].
    xg = nc.declare_dram_parameter("xg", [P, KH * cap], BF16, isOutput=False)
    wiT = nc.declare_dram_parameter("wiT", [P, KH * F], BF16, isOutput=False)
    # c-major: [P, c(6), j(24), col(128)] flattened
    woT = nc.declare_dram_parameter("woT", [P, KH * KF * P], BF16, isOutput=False)
    bibo = nc.declare_dram_parameter("bibo", [P, KF + KH], F32, isOutput=False)
    # transposed output: token t0+t of block ib lives at [p=h%128, KH*t0 +
    # k*b + t] per h-chunk k (block-major, like xg). Unweighted expert rows
    # in bf16; the host applies the fp32 routing weights during scatter-add.
    out = nc.declare_dram_parameter("out", [P, KH * cap], BF16, isOutput=True)
    # scratch sink: a gpsimd-issued DMA reading an hT slice paces the gpsimd
    # engine to that gelu, so its subsequent weight injects wait for compute.
    # (Only gpsimd/scalar may be paced this way — the sync engine services
    # the tile framework's semaphore chains and parking it stalls everything.)
    pace = nc.declare_dram_parameter("pace", [1, 8], BF16, isOutput=True)

    # j (0..23) -> (wi group tile index, local column slice)
    j_map = []
    for gi, gw in enumerate(WI_GROUPS):
        for jj in range(gw):
            j_map.append((gi, jj))
    goff = [sum(WI_GROUPS[:g]) for g in range(len(WI_GROUPS))]

    with tile.TileContext(nc) as tc:
        with (
            tc.tile_pool(name="weights", bufs=1) as wpool,
            tc.tile_pool(name="xin", bufs=3) as xpool,
            tc.tile_pool(name="hbuf", bufs=2) as hpool,
            tc.tile_pool(name="obuf", bufs=2) as opool,
            tc.tile_pool(name="psum_h", bufs=6, space="PSUM") as ph_pool,
            tc.tile_pool(name="psum_o", bufs=2, space="PSUM") as po_pool,
        ):
            def x_dma(eng, xt, t0, b):
                eng.dma_start(
                    out=xt,
                    in_=xg[:, KH * t0 : KH * (t0 + b)].rearrange(
                        "p (k t) -> p k t", k=KH
                    ),
                )

            def wig_dma(eng, wt, g):
                a = KH * P * goff[g]
                w = KH * P * WI_GROUPS[g]
                eng.dma_start(
                    out=wt,
                    in_=wiT[:, a : a + w].rearrange("p (k c) -> p k c", k=KH),
                )

            b0 = blocks[0]
            x_tiles = {}
            x0_bf = xpool.tile([P, KH, b0], BF16, tag="xb", name="x0_bf")
            x_tiles[0] = x0_bf
            wi_groups = [
                wpool.tile(
                    [P, KH, gw * P], BF16, tag=f"wig{gi}", name=f"wig{gi}"
                )
                for gi, gw in enumerate(WI_GROUPS)
            ]
            bibo_sb = wpool.tile([P, KF + KH], F32)
            wo_sb = wpool.tile([P, KH, KF * P], BF16)

            def wo_dma(eng, c0, c1):
                eng.dma_start(
                    out=wo_sb[:, c0:c1, :],
                    in_=woT[:, c0 * KF * P : c1 * KF * P].rearrange(
                        "p (c n) -> p c n", c=c1 - c0
                    ),
                )

            # ---- preamble DMAs: ONLY the phase-A critical window items,
            # interleaved across queues in deadline order (queue early rates:
            # sync/gpsimd ~100KB/us, scalar ~45KB/us). Everything else is
            # paced into the loop.
            x0a_src = xg[:, 0 : 3 * b0].rearrange("p (k t) -> p k t", k=3)
            nc.gpsimd.dma_start(
                out=x0_bf[64:128, 0:3, :], in_=x0a_src[64:128, :, :]
            )
            nc.sync.dma_start(
                out=x0_bf[0:64, 0:3, :], in_=x0a_src[0:64, :, :]
            )
            nc.scalar.dma_start(out=bibo_sb, in_=bibo[:, :])
            nc.gpsimd.dma_start(
                out=x0_bf[:, 3:6, :],
                in_=xg[:, 3 * b0 : 6 * b0].rearrange("p (k t) -> p k t", k=3),
            )
            wig_dma(nc.sync, wi_groups[0], 0)
            wig_dma(nc.sync, wi_groups[1], 1)
            wig_dma(nc.scalar, wi_groups[2], 2)
            wig_dma(nc.gpsimd, wi_groups[3], 3)
            wig_dma(nc.sync, wi_groups[4], 4)
            wig_dma(nc.scalar, wi_groups[5], 5)

            t0 = 0
            for ib, b in enumerate(blocks):
                last_blk = ib == nblk - 1

                x_bf = x_tiles.pop(ib)
                # prefetch block ib+2's x on gpsimd (idle engine; WAR on the
                # pool buffer naturally delays it past block ib's reads)
                if ib >= 1 and ib + 2 < nblk:
                    bn = blocks[ib + 2]
                    x_next = xpool.tile(
                        [P, KH, bn], BF16, tag="xb", name=f"x{ib + 2}_bf"
                    )
                    x_tiles[ib + 2] = x_next
                    x_dma(nc.gpsimd, x_next, sum(blocks[: ib + 2]), bn)

                # ---- layer 1: hT[f, t] = gelu(WiT^T @ xT + bi) ----
                hT = hpool.tile([P, KF, b], BF16, tag="hT")
                for j in range(KF):
                    gi, jj = j_map[j]
                    ps = ph_pool.tile([P, b], F32, tag="ph")
                    wig = wi_groups[gi]
                    for k in range(KH):
                        nc.tensor.matmul(
                            ps,
                            lhsT=wig[:, k, jj * P : (jj + 1) * P],
                            rhs=x_bf[:, k, :],
                            start=(k == 0),
                            stop=(k == KH - 1),
                        )
                    nc.scalar.activation(
                        out=hT[:, j, :],
                        in_=ps,
                        func=mybir.ActivationFunctionType.Gelu,
                        bias=bibo_sb[:, j : j + 1],
                        scale=1.0,
                    )
                    if ib == 0:
                        # paced phase-B injects. The pace DMA reads gelu 0's
                        # output, so the gpsimd engine (and with it the wi
                        # bulk injects on its queue) waits for compute to
                        # reach here; scalar paces naturally between gelus.
                        if j == 0:
                            nc.gpsimd.dma_start(
                                out=pace[:, :], in_=hT[0:1, 0, 0:8]
                            )
                            wig_dma(nc.gpsimd, wi_groups[6], 6)
                            wig_dma(nc.gpsimd, wi_groups[7], 7)
                        elif j == 2 and nblk > 1:
                            b1 = blocks[1]
                            x1_bf = xpool.tile(
                                [P, KH, b1], BF16, tag="xb", name="x1_bf"
                            )
                            x_tiles[1] = x1_bf
                            x_dma(nc.scalar, x1_bf, b0, b1)
                        elif j == 4:
                            wo_dma(nc.scalar, 0, 2)
                        elif j == 6:
                            wo_dma(nc.scalar, 2, 6)
                        elif j == 8 and nblk > 2:
                            b2 = blocks[2]
                            x2_bf = xpool.tile(
                                [P, KH, b2], BF16, tag="xb", name="x2_bf"
                            )
                            x_tiles[2] = x2_bf
                            x_dma(nc.scalar, x2_bf, blocks[0] + blocks[1], b2)

                # ---- layer 2 (transposed): outT[h, t] = WoT-chunk^T @ hT + bo.
                # Tokens are the matmul free dim, so a partial tail tile
                # costs proportionally, and bo is a per-partition scalar. ----
                o_blkT = opool.tile([P, KH, b], BF16, tag="os")
                for c in range(KH):
                    pc = po_pool.tile([P, b], F32, tag="po")
                    for j in range(KF):
                        nc.tensor.matmul(
                            pc,
                            lhsT=wo_sb[:, c, j * P : (j + 1) * P],
                            rhs=hT[:, j, :],
                            start=(j == 0),
                            stop=(j == KF - 1),
                        )
                    nc.vector.tensor_scalar(
                        o_blkT[:, c, :], pc,
                        scalar1=bibo_sb[:, KF + c : KF + c + 1],
                        scalar2=None, op0=mybir.AluOpType.add,
                    )
                if last_blk:
                    # final writes: chunk PAIRS as their epilogues complete,
                    # last pair partition-halved (64-ALIGNED — misaligned
                    # partition windows fragment into tiny packets) across
                    # the two fast queues so the tail drain is ~64 packets
                    for c in range(KH):
                        if c == 1:
                            nc.sync.dma_start(
                                out=out[:, KH * t0 : KH * t0 + 2 * b],
                                in_=o_blkT[:, 0:2, :],
                            )
                        elif c == 3:
                            nc.gpsimd.dma_start(
                                out=out[:, KH * t0 + 2 * b : KH * t0 + 4 * b],
                                in_=o_blkT[:, 2:4, :],
                            )
                        elif c == 5:
                            dst = out[:, KH * t0 + 4 * b : KH * t0 + 6 * b]
                            nc.sync.dma_start(
                                out=dst[0:64, :], in_=o_blkT[0:64, 4:6, :]
                            )
                            nc.gpsimd.dma_start(
                                out=dst[64:128, :], in_=o_blkT[64:128, 4:6, :]
                            )
                else:
                    nc.sync.dma_start(
                        out=out[:, KH * t0 : KH * (t0 + b)].rearrange(
                            "p (k t) -> p k t", k=KH
                        ),
                        in_=o_blkT,
                    )
                t0 += b

    nc.compile()
    return nc


_NC_CACHE: dict = {}


def _get_nc(cap: int):
    if cap not in _NC_CACHE:
        _NC_CACHE[cap] = build_nc(cap)
    return _NC_CACHE[cap]


def _ensure_axon_hooks_module():
    """run_bass_kernel_spmd(trace=True) (e.g. via env BASS_TRACE=1) imports
    antenv.axon_hooks, which some images lack even though the boot code that
    would register the NTFF hook is present. Provide the module and register
    the real hook when available so tracing works instead of crashing."""
    try:
        import antenv.axon_hooks  # noqa: F401

        return
    except ImportError:
        pass
    try:
        import sys
        import types

        import antenv  # noqa: F401

        mod = types.ModuleType("antenv.axon_hooks")
        state = {"hook": None}
        mod.set_axon_ntff_profile_hook = lambda h: state.__setitem__("hook", h)
        mod.get_axon_ntff_profile_hook = lambda: state["hook"]
        try:
            from trn_agent_boot.trn_boot import _ntff_profile_via_ctypes

            mod.set_axon_ntff_profile_hook(
                _ntff_profile_via_ctypes("/opt/axon/libaxon_pjrt.so")
            )
        except Exception:
            pass
        sys.modules["antenv.axon_hooks"] = mod
    except Exception:
        pass


def _route(xf, Wr):
    """Host router in fp32: top-2 expert indices (matching jax.lax.top_k
    tie-breaking: lowest index wins) and softmax weights over the top-2."""
    logits = xf.astype(np.float32) @ np.asarray(Wr, np.float32).T  # [T, E]
    i1 = np.argmax(logits, axis=1)
    l2 = logits.copy()
    rows = np.arange(len(i1))
    l2[rows, i1] = -np.inf
    i2 = np.argmax(l2, axis=1)
    m1 = logits[rows, i1]
    m2 = l2[rows, i2]
    e = np.exp(m2 - m1)
    w1 = 1.0 / (1.0 + e)
    w2 = e / (1.0 + e)
    tokens = np.arange(logits.shape[0])
    tok_lists, w_lists = [], []
    for c in range(N_CORES):
        tok_lists.append(np.concatenate([tokens[i1 == c], tokens[i2 == c]]))
        w_lists.append(np.concatenate([w1[i1 == c], w2[i2 == c]]))
    return tok_lists, w_lists


def _pack_kpf(a2d, k):
    """[k*128, N] row-major -> [128, k*N] partition-major (k-major per row)."""
    kk, n = a2d.shape
    assert kk == k * P
    return np.ascontiguousarray(
        a2d.reshape(k, P, n).transpose(1, 0, 2).reshape(P, k * n)
    )


def _pack_wi_groups(wiT2d):
    """[H, F] -> [128, KH*F] GROUP-major: each wi column group's
    [KH, group_cols] block is contiguous per partition."""
    v = wiT2d.reshape(KH, P, F)
    parts = []
    c0 = 0
    for gw in WI_GROUPS:
        parts.append(
            v[:, :, c0 : c0 + gw * P].transpose(1, 0, 2).reshape(P, KH * gw * P)
        )
        c0 += gw * P
    return np.ascontiguousarray(np.concatenate(parts, axis=1))


def _pack_wo_cmajor(woT2d):
    """[F, H] -> [128, KH*KF*128] c-major: per partition p (=f%128), layout
    [c][j][col] with element = WoT[j*128+p, c*128+col]."""
    v = woT2d.reshape(KF, P, KH, P)  # [j, p, c, col]
    return np.ascontiguousarray(
        v.transpose(1, 2, 0, 3).reshape(P, KH * KF * P)
    )


def kernel(x, Wr, Wi, bi, Wo, bo, _trace=False):
    x = np.asarray(x)
    xf = x.reshape(-1, H).astype(np.float32)
    tok_lists, w_lists = _route(xf, Wr)
    cap = max(512, max(len(tl) for tl in tok_lists))
    blocks = make_blocks(cap)

    xT = np.ascontiguousarray(xf.T).astype(BF16_NP)  # [H, T] bf16
    bi_full = np.asarray(bi, np.float32)
    bo_full = np.asarray(bo, np.float32)

    in_maps = []
    for c in range(N_CORES):
        tl = tok_lists[c]
        xg = np.zeros((H, cap), dtype=BF16_NP)
        xg[:, : len(tl)] = xT[:, tl]
        # block-major packing: [128, sum_b KH*b], block ib at offset KH*t0
        xg_k = xg.reshape(KH, P, cap)
        xg_p = np.empty((P, KH * cap), dtype=BF16_NP)
        t0 = 0
        for b in blocks:
            xg_p[:, KH * t0 : KH * (t0 + b)] = (
                xg_k[:, :, t0 : t0 + b].transpose(1, 0, 2).reshape(P, KH * b)
            )
            t0 += b
        bibo_c = np.concatenate(
            [
                _pack_kpf(bi_full[c].reshape(F, 1), KF).reshape(P, KF),
                _pack_kpf(bo_full[c].reshape(H, 1), KH).reshape(P, KH),
            ],
            axis=1,
        )
        in_maps.append(
            {
                "xg": xg_p,
                "wiT": _pack_wi_groups(
                    np.asarray(Wi[c], np.float32).T.astype(BF16_NP)
                ),
                "woT": _pack_wo_cmajor(
                    np.ascontiguousarray(np.asarray(Wo[c], np.float32).T).astype(
                        BF16_NP
                    )
                ),
                "bibo": bibo_c,
            }
        )

    _ensure_axon_hooks_module()
    nc = _get_nc(cap)
    res = run_bass_kernel_spmd(
        nc, in_maps, core_ids=list(range(N_CORES)), trace=_trace
    )

    # Unshard: scatter-add each core's expert rows scaled by its fp32
    # routing weight.
    out = np.zeros((T, H), dtype=np.float32)
    for c in range(N_CORES):
        tl = tok_lists[c]
        n = len(tl)
        # out param is [128, KH*cap] block-major: token t0+t of block ib at
        # [p, KH*t0 + k*b + t] -> rows h = k*128+p
        o = np.asarray(res.results[c]["out"]).astype(np.float32)  # [P, KH*cap]
        o_rows = np.empty((n, H), dtype=np.float32)
        t0 = 0
        for b in blocks:
            if t0 >= n:
                break
            m = min(b, n - t0)
            blk = o[:, KH * t0 : KH * (t0 + b)].reshape(P, KH, b)
            # [P, KH, b] -> tokens t0..t0+m, features h=k*128+p
            o_rows[t0 : t0 + m] = blk[:, :, :m].transpose(2, 1, 0).reshape(m, H)
            t0 += b
        out[tl] += w_lists[c][:n, None] * o_rows
    out = out.reshape(x.shape)
    if _trace:
        return out, res
    return out


# revision 38
# speedup vs baseline: 1.0119x; 1.0119x over previous
"""BertMoELayer (B=4, S=2048, H=768, F=3072, E=8, top-2) on 8 Trainium2 cores.

Expert-parallel: one expert per core. The host evaluates the router in fp32
(it must anyway, to decide the shard assignment matching jax.lax.top_k
tie-breaking) and also produces the top-2 softmax weights there — they are
O(T) scalars and bit-match the reference combine. The device runs the dense
FFN only:

  per core c, over its gathered tokens (capacity = max expert load, exact):
    hT      = gelu(WiT^T @ xT + bi[c])        (bf16 matmul, fp32 psum)
    outT_c  = WoT-chunk^T @ hT + bo[c]        (bf16 matmul, fp32 psum, bf16 out;
                                               [h, token] orientation so the
                                               tail tokens cost proportionally)

The host unshards by scatter-adding each core's expert rows scaled by its
fp32 routing weight.

Startup and teardown are the only non-roofline phases (steady state measures
at the matmul issue-rate roofline, ~216ns per 512-token matmul), so the DMA
schedule is built around the measured DGE behavior:
  - DMAs sharing a queue transfer CONCURRENTLY, bandwidth split roughly pro
    packet size with a FIFO bias — so big-packet bulk co-resident with small
    critical loads starves them;
  - per-queue early throughput is ~100KB/us (sync/gpsimd) and ~45KB/us
    (scalar), rising substantially after ~25us;
  - each DMA_DIRECT2D inject costs ~0.7us of the ISSUING engine, and the
    issuing engine's queue is the one that carries the transfer.
Hence:
  - the preamble injects ONLY the phase-A critical window: x block 0 in two
    k-pieces on gpsimd, the first wi groups (1,2,2,2,2,2 cols*128)
    interleaved across sync/scalar/gpsimd in consumption-deadline order;
  - the wi bulk (6+7 cols*128) is PACED: a dummy DMA reading gelu 0's output
    parks the gpsimd engine until compute reaches it, so the bulk's
    transfers only hit the queue after the critical items have landed
    (pacing this way is legal on gpsimd/scalar only — the sync engine
    services the tile framework's semaphore chains; parking it stalls all);
  - wo is packed C-MAJOR (output-chunk-major) and paced off the scalar
    engine between early gelus, so layer 2 starts on a partial wo stream;
  - the final block's output leaves as chunk-pairs on sync/gpsimd with the
    last pair partition-halved 64/64 (64-ALIGNED — misaligned partition
    windows fragment into tiny packets), since the tail drain is
    packet-rate-limited.

All tensors are HOST-PREPACKED into SBUF-partition-major layout ([128, ...]
with each partition's bytes contiguous in DRAM): every DMA moves 1.5-24KB
contiguous lines per partition.
"""

import numpy as np
import ml_dtypes

import concourse.tile as tile
from concourse import bacc, mybir
from concourse.bass_utils import run_bass_kernel_spmd

B, S, H, F, E = 4, 2048, 768, 3072, 8
T = B * S
N_CORES = 8

P = 128          # SBUF partitions
KH = H // P      # 6   h-chunks
KF = F // P      # 24  f-chunks

F32 = mybir.dt.float32
BF16 = mybir.dt.bfloat16
BF16_NP = ml_dtypes.bfloat16

# wi column groups (in j units of 128): small groups interleaved across all
# three DGE queues in deadline order at startup (DMAs sharing a queue
# transfer CONCURRENTLY, pro packet size — big-packet bulk must not
# co-reside with these); the 6+7 bulk is paced onto the gpsimd queue
# mid-loop once gelu 0 has run.
WI_GROUPS = (1, 2, 2, 2, 2, 2, 6, 7)


def make_blocks(cap: int):
    """Token blocks: 512-token blocks (psum-bank-sized) with a 257..512
    tail; every block >= 256 tokens so L1 chains stay matmul-bound (a
    512-free bf16 matmul is 213ns vs ~97ns LDWEIGHTS)."""
    assert cap >= 512
    blocks = []
    rem = cap
    while rem > 768:
        blocks.append(512)
        rem -= 512
    if rem > 512:
        blocks.append(256)
        rem -= 256
    blocks.append(rem)
    assert sum(blocks) == cap
    assert all(b % 128 == 0 for b in blocks[:-1]) and blocks[-1] <= 512
    return blocks


def build_nc(cap: int):
    """Per-core program: dense expert FFN over `cap` tokens."""
    blocks = make_blocks(cap)
    nblk = len(blocks)

    # Bacc (not plain Bass): its compile() pass splits multi-wait instructions
    # into event-semaphore chains, which walrus requires (max 1 wait per inst).
    nc = bacc.Bacc(None)

    # All inputs prepacked on host to [128 partitions, contiguous bytes].
    xg = nc.declare_dram_parameter("xg", [P, KH * cap], BF16, isOutput=False)
    wiT = nc.declare_dram_parameter("wiT", [P, KH * F], BF16, isOutput=False)
    # c-major: [P, c(6), j(24), col(128)] flattened
    woT = nc.declare_dram_parameter("woT", [P, KH * KF * P], BF16, isOutput=False)
    bibo = nc.declare_dram_parameter("bibo", [P, KF + KH], F32, isOutput=False)
    # transposed output: token t0+t of block ib lives at [p=h%128, KH*t0 +
    # k*b + t] per h-chunk k (block-major, like xg). Unweighted expert rows
    # in bf16; the host applies the fp32 routing weights during scatter-add.
    out = nc.declare_dram_parameter("out", [P, KH * cap], BF16, isOutput=True)
    # scratch sink: a gpsimd-issued DMA reading an hT slice paces the gpsimd
    # engine to that gelu, so its subsequent weight injects wait for compute.
    # (Only gpsimd/scalar may be paced this way — the sync engine services
    # the tile framework's semaphore chains and parking it stalls everything.)
    pace = nc.declare_dram_parameter("pace", [1, 8], BF16, isOutput=True)

    # j (0..23) -> (wi group tile index, local column slice)
    j_map = []
    for gi, gw in enumerate(WI_GROUPS):
        for jj in range(gw):
            j_map.append((gi, jj))
    goff = [sum(WI_GROUPS[:g]) for g in range(len(WI_GROUPS))]

    with tile.TileContext(nc) as tc:
        with (
            tc.tile_pool(name="weights", bufs=1) as wpool,
            tc.tile_pool(name="xin", bufs=3) as xpool,
            tc.tile_pool(name="hbuf", bufs=2) as hpool,
            tc.tile_pool(name="obuf", bufs=2) as opool,
            tc.tile_pool(name="psum_h", bufs=6, space="PSUM") as ph_pool,
            tc.tile_pool(name="psum_o", bufs=2, space="PSUM") as po_pool,
        ):
            def x_dma(eng, xt, t0, b):
                eng.dma_start(
                    out=xt,
                    in_=xg[:, KH * t0 : KH * (t0 + b)].rearrange(
                        "p (k t) -> p k t", k=KH
                    ),
                )

            def wig_dma(eng, wt, g):
                a = KH * P * goff[g]
                w = KH * P * WI_GROUPS[g]
                eng.dma_start(
                    out=wt,
                    in_=wiT[:, a : a + w].rearrange("p (k c) -> p k c", k=KH),
                )

            b0 = blocks[0]
            x_tiles = {}
            x0_bf = xpool.tile([P, KH, b0], BF16, tag="xb", name="x0_bf")
            x_tiles[0] = x0_bf
            wi_groups = [
                wpool.tile(
                    [P, KH, gw * P], BF16, tag=f"wig{gi}", name=f"wig{gi}"
                )
                for gi, gw in enumerate(WI_GROUPS)
            ]
            bibo_sb = wpool.tile([P, KF + KH], F32)
            wo_sb = wpool.tile([P, KH, KF * P], BF16)

            def wo_dma(eng, c0, c1):
                eng.dma_start(
                    out=wo_sb[:, c0:c1, :],
                    in_=woT[:, c0 * KF * P : c1 * KF * P].rearrange(
                        "p (c n) -> p c n", c=c1 - c0
                    ),
                )

            # ---- preamble DMAs: ONLY the phase-A critical window items,
            # interleaved across queues in deadline order (queue early rates:
            # sync/gpsimd ~100KB/us, scalar ~45KB/us). Everything else is
            # paced into the loop.
            nc.gpsimd.dma_start(
                out=x0_bf[:, 0:3, :],
                in_=xg[:, 0 : 3 * b0].rearrange("p (k t) -> p k t", k=3),
            )
            wig_dma(nc.sync, wi_groups[0], 0)
            nc.scalar.dma_start(out=bibo_sb, in_=bibo[:, :])
            nc.gpsimd.dma_start(
                out=x0_bf[:, 3:6, :],
                in_=xg[:, 3 * b0 : 6 * b0].rearrange("p (k t) -> p k t", k=3),
            )
            wig_dma(nc.sync, wi_groups[1], 1)
            wig_dma(nc.scalar, wi_groups[2], 2)
            wig_dma(nc.gpsimd, wi_groups[3], 3)
            wig_dma(nc.sync, wi_groups[4], 4)
            wig_dma(nc.scalar, wi_groups[5], 5)

            t0 = 0
            for ib, b in enumerate(blocks):
                last_blk = ib == nblk - 1

                x_bf = x_tiles.pop(ib)
                # prefetch block ib+2's x on gpsimd (idle engine; WAR on the
                # pool buffer naturally delays it past block ib's reads)
                if ib >= 1 and ib + 2 < nblk:
                    bn = blocks[ib + 2]
                    x_next = xpool.tile(
                        [P, KH, bn], BF16, tag="xb", name=f"x{ib + 2}_bf"
                    )
                    x_tiles[ib + 2] = x_next
                    x_dma(nc.gpsimd, x_next, sum(blocks[: ib + 2]), bn)

                # ---- layer 1: hT[f, t] = gelu(WiT^T @ xT + bi) ----
                hT = hpool.tile([P, KF, b], BF16, tag="hT")
                for j in range(KF):
                    gi, jj = j_map[j]
                    ps = ph_pool.tile([P, b], F32, tag="ph")
                    wig = wi_groups[gi]
                    for k in range(KH):
                        nc.tensor.matmul(
                            ps,
                            lhsT=wig[:, k, jj * P : (jj + 1) * P],
                            rhs=x_bf[:, k, :],
                            start=(k == 0),
                            stop=(k == KH - 1),
                        )
                    nc.scalar.activation(
                        out=hT[:, j, :],
                        in_=ps,
                        func=mybir.ActivationFunctionType.Gelu,
                        bias=bibo_sb[:, j : j + 1],
                        scale=1.0,
                    )
                    if ib == 0:
                        # paced phase-B injects. The pace DMA reads gelu 0's
                        # output, so the gpsimd engine (and with it the wi
                        # bulk injects on its queue) waits for compute to
                        # reach here; scalar paces naturally between gelus.
                        if j == 0:
                            nc.gpsimd.dma_start(
                                out=pace[:, :], in_=hT[0:1, 0, 0:8]
                            )
                            wig_dma(nc.gpsimd, wi_groups[6], 6)
                            wig_dma(nc.gpsimd, wi_groups[7], 7)
                        elif j == 2 and nblk > 1:
                            b1 = blocks[1]
                            x1_bf = xpool.tile(
                                [P, KH, b1], BF16, tag="xb", name="x1_bf"
                            )
                            x_tiles[1] = x1_bf
                            x_dma(nc.scalar, x1_bf, b0, b1)
                        elif j == 4:
                            wo_dma(nc.scalar, 0, 2)
                        elif j == 6:
                            wo_dma(nc.scalar, 2, 6)
                        elif j == 8 and nblk > 2:
                            b2 = blocks[2]
                            x2_bf = xpool.tile(
                                [P, KH, b2], BF16, tag="xb", name="x2_bf"
                            )
                            x_tiles[2] = x2_bf
                            x_dma(nc.scalar, x2_bf, blocks[0] + blocks[1], b2)

                # ---- layer 2 (transposed): outT[h, t] = WoT-chunk^T @ hT + bo.
                # Tokens are the matmul free dim, so a partial tail tile
                # costs proportionally, and bo is a per-partition scalar. ----
                o_blkT = opool.tile([P, KH, b], BF16, tag="os")
                for c in range(KH):
                    pc = po_pool.tile([P, b], F32, tag="po")
                    for j in range(KF):
                        nc.tensor.matmul(
                            pc,
                            lhsT=wo_sb[:, c, j * P : (j + 1) * P],
                            rhs=hT[:, j, :],
                            start=(j == 0),
                            stop=(j == KF - 1),
                        )
                    nc.vector.tensor_scalar(
                        o_blkT[:, c, :], pc,
                        scalar1=bibo_sb[:, KF + c : KF + c + 1],
                        scalar2=None, op0=mybir.AluOpType.add,
                    )
                if last_blk:
                    # final writes: chunk PAIRS as their epilogues complete,
                    # last pair partition-halved (64-ALIGNED — misaligned
                    # partition windows fragment into tiny packets) across
                    # the two fast queues so the tail drain is ~64 packets
                    for c in range(KH):
                        if c == 1:
                            nc.sync.dma_start(
                                out=out[:, KH * t0 : KH * t0 + 2 * b],
                                in_=o_blkT[:, 0:2, :],
                            )
                        elif c == 3:
                            nc.gpsimd.dma_start(
                                out=out[:, KH * t0 + 2 * b : KH * t0 + 4 * b],
                                in_=o_blkT[:, 2:4, :],
                            )
                        elif c == 5:
                            dst = out[:, KH * t0 + 4 * b : KH * t0 + 6 * b]
                            nc.sync.dma_start(
                                out=dst[0:64, :], in_=o_blkT[0:64, 4:6, :]
                            )
                            nc.gpsimd.dma_start(
                                out=dst[64:128, :], in_=o_blkT[64:128, 4:6, :]
                            )
                else:
                    nc.sync.dma_start(
                        out=out[:, KH * t0 : KH * (t0 + b)].rearrange(
                            "p (k t) -> p k t", k=KH
                        ),
                        in_=o_blkT,
                    )
                t0 += b

    nc.compile()
    return nc


_NC_CACHE: dict = {}


def _get_nc(cap: int):
    if cap not in _NC_CACHE:
        _NC_CACHE[cap] = build_nc(cap)
    return _NC_CACHE[cap]


def _ensure_axon_hooks_module():
    """run_bass_kernel_spmd(trace=True) (e.g. via env BASS_TRACE=1) imports
    antenv.axon_hooks, which some images lack even though the boot code that
    would register the NTFF hook is present. Provide the module and register
    the real hook when available so tracing works instead of crashing."""
    try:
        import antenv.axon_hooks  # noqa: F401

        return
    except ImportError:
        pass
    try:
        import sys
        import types

        import antenv  # noqa: F401

        mod = types.ModuleType("antenv.axon_hooks")
        state = {"hook": None}
        mod.set_axon_ntff_profile_hook = lambda h: state.__setitem__("hook", h)
        mod.get_axon_ntff_profile_hook = lambda: state["hook"]
        try:
            from trn_agent_boot.trn_boot import _ntff_profile_via_ctypes

            mod.set_axon_ntff_profile_hook(
                _ntff_profile_via_ctypes("/opt/axon/libaxon_pjrt.so")
            )
        except Exception:
            pass
        sys.modules["antenv.axon_hooks"] = mod
    except Exception:
        pass


def _route(xf, Wr):
    """Host router in fp32: top-2 expert indices (matching jax.lax.top_k
    tie-breaking: lowest index wins) and softmax weights over the top-2."""
    logits = xf.astype(np.float32) @ np.asarray(Wr, np.float32).T  # [T, E]
    i1 = np.argmax(logits, axis=1)
    l2 = logits.copy()
    rows = np.arange(len(i1))
    l2[rows, i1] = -np.inf
    i2 = np.argmax(l2, axis=1)
    m1 = logits[rows, i1]
    m2 = l2[rows, i2]
    e = np.exp(m2 - m1)
    w1 = 1.0 / (1.0 + e)
    w2 = e / (1.0 + e)
    tokens = np.arange(logits.shape[0])
    tok_lists, w_lists = [], []
    for c in range(N_CORES):
        tok_lists.append(np.concatenate([tokens[i1 == c], tokens[i2 == c]]))
        w_lists.append(np.concatenate([w1[i1 == c], w2[i2 == c]]))
    return tok_lists, w_lists


def _pack_kpf(a2d, k):
    """[k*128, N] row-major -> [128, k*N] partition-major (k-major per row)."""
    kk, n = a2d.shape
    assert kk == k * P
    return np.ascontiguousarray(
        a2d.reshape(k, P, n).transpose(1, 0, 2).reshape(P, k * n)
    )


def _pack_wi_groups(wiT2d):
    """[H, F] -> [128, KH*F] GROUP-major: each wi column group's
    [KH, group_cols] block is contiguous per partition."""
    v = wiT2d.reshape(KH, P, F)
    parts = []
    c0 = 0
    for gw in WI_GROUPS:
        parts.append(
            v[:, :, c0 : c0 + gw * P].transpose(1, 0, 2).reshape(P, KH * gw * P)
        )
        c0 += gw * P
    return np.ascontiguousarray(np.concatenate(parts, axis=1))


def _pack_wo_cmajor(woT2d):
    """[F, H] -> [128, KH*KF*128] c-major: per partition p (=f%128), layout
    [c][j][col] with element = WoT[j*128+p, c*128+col]."""
    v = woT2d.reshape(KF, P, KH, P)  # [j, p, c, col]
    return np.ascontiguousarray(
        v.transpose(1, 2, 0, 3).reshape(P, KH * KF * P)
    )


def kernel(x, Wr, Wi, bi, Wo, bo, _trace=False):
    x = np.asarray(x)
    xf = x.reshape(-1, H).astype(np.float32)
    tok_lists, w_lists = _route(xf, Wr)
    cap = max(512, max(len(tl) for tl in tok_lists))
    blocks = make_blocks(cap)

    xT = np.ascontiguousarray(xf.T).astype(BF16_NP)  # [H, T] bf16
    bi_full = np.asarray(bi, np.float32)
    bo_full = np.asarray(bo, np.float32)

    in_maps = []
    for c in range(N_CORES):
        tl = tok_lists[c]
        xg = np.zeros((H, cap), dtype=BF16_NP)
        xg[:, : len(tl)] = xT[:, tl]
        # block-major packing: [128, sum_b KH*b], block ib at offset KH*t0
        xg_k = xg.reshape(KH, P, cap)
        xg_p = np.empty((P, KH * cap), dtype=BF16_NP)
        t0 = 0
        for b in blocks:
            xg_p[:, KH * t0 : KH * (t0 + b)] = (
                xg_k[:, :, t0 : t0 + b].transpose(1, 0, 2).reshape(P, KH * b)
            )
            t0 += b
        bibo_c = np.concatenate(
            [
                _pack_kpf(bi_full[c].reshape(F, 1), KF).reshape(P, KF),
                _pack_kpf(bo_full[c].reshape(H, 1), KH).reshape(P, KH),
            ],
            axis=1,
        )
        in_maps.append(
            {
                "xg": xg_p,
                "wiT": _pack_wi_groups(
                    np.asarray(Wi[c], np.float32).T.astype(BF16_NP)
                ),
                "woT": _pack_wo_cmajor(
                    np.ascontiguousarray(np.asarray(Wo[c], np.float32).T).astype(
                        BF16_NP
                    )
                ),
                "bibo": bibo_c,
            }
        )

    _ensure_axon_hooks_module()
    nc = _get_nc(cap)
    res = run_bass_kernel_spmd(
        nc, in_maps, core_ids=list(range(N_CORES)), trace=_trace
    )

    # Unshard: scatter-add each core's expert rows scaled by its fp32
    # routing weight.
    out = np.zeros((T, H), dtype=np.float32)
    for c in range(N_CORES):
        tl = tok_lists[c]
        n = len(tl)
        # out param is [128, KH*cap] block-major: token t0+t of block ib at
        # [p, KH*t0 + k*b + t] -> rows h = k*128+p
        o = np.asarray(res.results[c]["out"]).astype(np.float32)  # [P, KH*cap]
        o_rows = np.empty((n, H), dtype=np.float32)
        t0 = 0
        for b in blocks:
            if t0 >= n:
                break
            m = min(b, n - t0)
            blk = o[:, KH * t0 : KH * (t0 + b)].reshape(P, KH, b)
            # [P, KH, b] -> tokens t0..t0+m, features h=k*128+p
            o_rows[t0 : t0 + m] = blk[:, :, :m].transpose(2, 1, 0).reshape(m, H)
            t0 += b
        out[tl] += w_lists[c][:n, None] * o_rows
    out = out.reshape(x.shape)
    if _trace:
        return out, res
    return out
